# revision 44
# baseline (speedup 1.0000x reference)
"""CrossAttention kernel for 8 Trainium2 NeuronCores.

Problem (hardcoded): B=8, SQ=SK=1024, Q_DIM=2048, KV_DIM=1024, E_DIM=2048,
H=16 heads, HD=128.  out = softmax((X_q Wq^T + bq)(X_k Wk^T + bk)^T / sqrt(HD))
                            @ (X_v Wv^T + bv) @ Wo^T + bo

Sharding: data-parallel over batch - each of the 8 cores computes one batch
element end-to-end; no collectives.

Per-core dataflow: the four big projections (q/k/v/out) run as fp8-e4m3
DoubleRow matmuls with hi+lo residual splitting: every operand X is stored
as X_hi = fp8(X), X_lo = fp8(X - X_hi), and each pair of 128-row
contraction blocks is covered by 3 DoubleRow instructions
  (W_hi.X_hi), (W_lo.X_hi), (W_hi.X_lo)   [W_lo.X_lo ~ 0.1% dropped]
giving 0.75x the bf16 cycle count at bf16-grade accuracy.  Host-side
operands (x inputs, all weights) are split for free; `ao` is split on
device after the PV transpose.  scores and PV stay fp16 (contraction is
only HD=128 there, DoubleRow pairing has nothing to pair).

Power-of-2 pre-scales keep every fp8 tensor's magnitude in e4m3's normal
range; they are undone via ACT scale params, the PV ones-column, and the
final output copy scale:
  wq' = Wq^T * HD^-0.5 * 2^9   -> qT = psum * 2^-9 + bq * HD^-0.5
  wk' = Wk^T * 2^5             -> kT = psum * 2^-5 + bk
  wv' = Wv^T * 2^5             -> v4 holds v * 32 (fp16)
  ones column = 2.0            -> ao = 16 * pv / sum(p)   (fp8-friendly)
  wo' = Wo^T * 2^6             -> out = psum * 2^-10
bv is folded into bo on the host (softmax rows sum to 1).

Software pipeline: iter h computes scores pairs (h,2..6)+(h+1,0)
interleaved with projections for head h+1 (one-iteration lead; the
shifted score schedule keeps the ACT exp drain ahead of pv(h), which
keeps the PE busy and avoids cost-model p-state ramp resets), then
PV(h) -> ao -> fp16 DMA-transpose -> fp8 hi/lo quantize into aoT8.
The prologue streams only wk0/xk/wq0/xq (the DMA device is one serial
resource) and computes head-0 q/k projections pair-outer behind the
stream; xv/wv and all of vproj group 0 live in iter 0, whose head-1
projections reuse weight slice 0 and need no new DMA.  Iters 14/15 pull
in out-projection accumulation (chunks st=4,5 of pass 0, pairs 0..5) to
hide the exp drain; the epilogue runs the remaining out-projection with
streamed Wo pair-tiles and staggered chunk drains.
"""

import sys

sys.path.insert(0, "/opt/trn_rl_repo")

import numpy as np
import ml_dtypes

import concourse.tile as tile
from concourse import bacc
import concourse.mybir as mybir
from concourse.bass_utils import run_bass_kernel_spmd

F32 = mybir.dt.float32
BF16 = mybir.dt.bfloat16
FP16 = mybir.dt.float16
FP8 = mybir.dt.float8e4
DR = mybir.MatmulPerfMode.DoubleRow
ACT_IDENT = mybir.ActivationFunctionType.Identity
ACT_COPY = mybir.ActivationFunctionType.Copy
ACT_EXP = mybir.ActivationFunctionType.Exp
SUB = mybir.AluOpType.subtract

B = 8
S = 1024          # SQ == SK
DQ = 2048         # query input dim
DKV = 1024        # key/value input dim
E = 2048          # embed dim
H = 16            # heads
HD = 128          # head dim
NT_S = S // 128   # 8 seq tiles
NT_E = E // 128   # 16 e tiles (== heads)
NT_DQ = DQ // 128
NT_DKV = DKV // 128
NP_DQ = NT_DQ // 2   # 8 contraction block-pairs
NP_DKV = NT_DKV // 2  # 4
NP_E = NT_E // 2     # 8
VROW = HD + 1     # head block in v group incl. ones column

# power-of-2 scale folding (see module docstring)
QS = 2.0 ** -9
KS = 2.0 ** -5
ONES_V = 2.0
OS = 2.0 ** -10
# the 3 DoubleRow terms per contraction block-pair: (w_j, x_j) digit picks
TERMS = ((0, 0), (1, 0), (0, 1))

_CACHED = {}
_DEV_EPI_PASSES = 4   # dev knob: number of epilogue passes to emit
_DEV_HEADS = H        # dev knob: number of main-loop head iterations


def _build():
    nc = bacc.Bacc("TRN2", target_bir_lowering=False, debug=False)

    xq8 = nc.dram_tensor("xq8", [2 * DQ, S], FP8, kind="ExternalInput")
    xk8 = nc.dram_tensor("xk8", [2 * DKV, S], FP8, kind="ExternalInput")
    xv8 = nc.dram_tensor("xv8", [2 * DKV, S], FP8, kind="ExternalInput")
    # weights arrive pre-sliced in partition-major layout so each slice
    # load is one full-rate DMA (>=512B contiguous per partition)
    wq8 = nc.dram_tensor("wq8", [8 * 128, NT_DQ * 2 * 256], FP8,
                         kind="ExternalInput")
    wk8 = nc.dram_tensor("wk8", [8 * 128, NT_DKV * 2 * 256], FP8,
                         kind="ExternalInput")
    wv8 = nc.dram_tensor("wv8", [4 * 128, NT_DKV * 2 * 512], FP8,
                         kind="ExternalInput")
    wo8 = nc.dram_tensor("wo8", [2 * E, E], FP8, kind="ExternalInput")
    bq = nc.dram_tensor("bq", [E], F32, kind="ExternalInput")
    bk = nc.dram_tensor("bk", [E], F32, kind="ExternalInput")
    out = nc.dram_tensor("out", [S, E], BF16, kind="ExternalOutput")

    xq_r = xq8.rearrange("(t j p) s -> p t j s", p=128, j=2)
    xk_r = xk8.rearrange("(t j p) s -> p t j s", p=128, j=2)
    xv_r = xv8.rearrange("(t j p) s -> p t j s", p=128, j=2)
    wq_r = wq8.rearrange("(sl p) (t jj c) -> sl p t jj c",
                         p=128, jj=2, c=256)
    wk_r = wk8.rearrange("(sl p) (t jj c) -> sl p t jj c",
                         p=128, jj=2, c=256)
    wv_r = wv8.rearrange("(sl p) (t jj c) -> sl p t jj c",
                         p=128, jj=2, c=512)
    wo_r = wo8.rearrange("(t j p) e -> p t j e", p=128, j=2)

    with tile.TileContext(nc) as tc:
        with (
            tc.tile_pool(name="persist", bufs=1) as persist,
            tc.tile_pool(name="qk", bufs=2) as qkp,
            tc.tile_pool(name="v4p", bufs=2) as v4p,
            tc.tile_pool(name="pts", bufs=10) as pts,
            tc.tile_pool(name="aohp", bufs=2) as aohp,
            tc.tile_pool(name="aostp", bufs=2) as aostp,
            tc.tile_pool(name="wqp", bufs=2) as wqp,
            tc.tile_pool(name="wkp", bufs=2) as wkp,
            tc.tile_pool(name="wvp", bufs=2) as wvp,
            tc.tile_pool(name="wop", bufs=7) as wop,
            tc.tile_pool(name="outsb", bufs=4) as outsb,
            tc.tile_pool(name="small", bufs=4) as small,
            tc.tile_pool(name="ssps", bufs=2, space="PSUM") as ssps,
            tc.tile_pool(name="paps", bufs=2, space="PSUM") as paps,
            tc.tile_pool(name="opps", bufs=2, space="PSUM") as opps,
        ):
            # ---- resident inputs / constants ----
            bq_sb = persist.tile([128, NT_E], F32, tag="bq")
            bk_sb = persist.tile([128, NT_E], F32, tag="bk")
            nc.gpsimd.dma_start(out=bq_sb, in_=bq.rearrange("(t p) -> p t", p=128))
            nc.gpsimd.dma_start(out=bk_sb, in_=bk.rearrange("(t p) -> p t", p=128))

            xq_sb = persist.tile([128, NT_DQ, 2, S], FP8, tag="xq")
            xk_sb = persist.tile([128, NT_DKV, 2, S], FP8, tag="xk")
            xv_sb = persist.tile([128, NT_DKV, 2, S], FP8, tag="xv")
            aoT8 = persist.tile([128, NT_E, 2, S], FP8, tag="aoT8")

            wq_sl = {}
            wk_sl = {}
            wv_sl = {}

            def load_wqk2(j, eng=None):  # heads 2j, 2j+1
                eng = eng or nc.scalar
                wq_sl[j] = wqp.tile([128, NT_DQ, 2, 256], FP8, tag="wq",
                                    name=f"wq2_{j}")
                eng.dma_start(out=wq_sl[j], in_=wq_r[j])
                wk_sl[j] = wkp.tile([128, NT_DKV, 2, 256], FP8, tag="wk",
                                    name=f"wk2_{j}")
                eng.dma_start(out=wk_sl[j], in_=wk_r[j])

            def load_wv(g, eng=None):  # heads 4g..4g+3
                eng = eng or nc.scalar
                wv_sl[g] = wvp.tile([128, NT_DKV, 2, 512], FP8, tag="wv",
                                    name=f"wv_{g}")
                eng.dma_start(out=wv_sl[g], in_=wv_r[g])

            # Prologue DMAs: only what the prologue compute needs (the DMA
            # device is a single serial resource in practice) — wk0, xk,
            # wq0, xq.  xv/wv stream during iter 0, whose vproj slot runs
            # all of group 0.  Input streams split across sync (even t) and
            # gpsimd (odd t) queues in consumption order; weight slices on
            # the scalar queue.
            wk_sl[0] = wkp.tile([128, NT_DKV, 2, 256], FP8, tag="wk",
                                name="wk2_0")
            nc.scalar.dma_start(out=wk_sl[0][:, 0:4], in_=wk_r[0][:, 0:4])
            nc.scalar.dma_start(out=wk_sl[0][:, 4:8], in_=wk_r[0][:, 4:8])

            def stream_x(sb, r_, nt):
                for t0 in range(0, nt, 4):
                    nc.sync.dma_start(out=sb[:, t0:t0 + 4],
                                      in_=r_[:, t0:t0 + 4])

            stream_x(xk_sb, xk_r, NT_DKV)
            wq_sl[0] = wqp.tile([128, NT_DQ, 2, 256], FP8, tag="wq",
                                name="wq2_0")
            nc.scalar.dma_start(out=wq_sl[0][:, 0:8], in_=wq_r[0][:, 0:8])
            nc.scalar.dma_start(out=wq_sl[0][:, 8:16], in_=wq_r[0][:, 8:16])
            stream_x(xq_sb, xq_r, NT_DQ)
            wo_pre = []

            qT = {}
            kT = {}
            v4 = {}
            _CARRY = {}

            def dr_chain(ps, stat_fn, mov_fn, npair):
                """48/24-instr 3-term DoubleRow chain into psum [128,512].

                stat_fn(tt, j) / mov_fn(tt, j, half) give the operand APs.
                One accumulation group: start on the first instr, stop on
                the last (the 2KB psum region lazily zeroes, so the second
                256-col half accumulates from clean zero).
                """
                n_tot = 2 * npair * 3
                n = 0
                for half in range(2):
                    for u in range(npair):
                        tt = slice(2 * u, 2 * u + 2)
                        for wj, xj in TERMS:
                            nc.tensor.matmul(
                                ps[:, half * 256:(half + 1) * 256],
                                stat_fn(tt, wj),
                                mov_fn(tt, xj, half),
                                start=(n == 0),
                                stop=(n == n_tot - 1),
                                perf_mode=DR,
                            )
                            n += 1

            def qproj(h):
                """qT[h] [e128, s] fp16 <- fp8 DR 3-term over DQ."""
                sl = wq_sl[h // 2]
                hoff = (h % 2) * 128
                qT[h] = qkp.tile([128, S], FP16, tag="qT", name=f"qT_{h}")
                for c in range(2):
                    ps = paps.tile([128, 512], F32, tag="pa", name=f"qps_{h}_{c}")
                    dr_chain(
                        ps,
                        lambda tt, wj: sl[:, tt, wj, hoff:hoff + 128],
                        lambda tt, xj, half: xq_sb[
                            :, tt, xj, c * 512 + half * 256:c * 512 + half * 256 + 256],
                        NP_DQ,
                    )
                    nc.scalar.activation(
                        out=qT[h][:, c * 512:(c + 1) * 512], in_=ps,
                        func=ACT_IDENT, bias=bq_sb[:, h:h + 1], scale=QS)

            def kproj(h):
                sl = wk_sl[h // 2]
                hoff = (h % 2) * 128
                kT[h] = qkp.tile([128, S], FP16, tag="kT", name=f"kT_{h}")
                for c in range(2):
                    ps = paps.tile([128, 512], F32, tag="pa", name=f"kps_{h}_{c}")
                    dr_chain(
                        ps,
                        lambda tt, wj: sl[:, tt, wj, hoff:hoff + 128],
                        lambda tt, xj, half: xk_sb[
                            :, tt, xj, c * 512 + half * 256:c * 512 + half * 256 + 256],
                        NP_DKV,
                    )
                    nc.scalar.activation(
                        out=kT[h][:, c * 512:(c + 1) * 512], in_=ps,
                        func=ACT_IDENT, bias=bk_sb[:, h:h + 1], scale=KS)

            def v4_alloc(g):
                v4[g] = v4p.tile([128, NT_S, 4 * VROW], FP16, tag="v4",
                                 name=f"v4_{g}")
                nc.vector.memset(
                    v4[g].rearrange("p t (h c) -> p t h c", c=VROW)
                    [:, :, :, HD:], ONES_V)

            def vproj_part(g, st0, nst):
                """v4[g][:, st, :] <- DR 3-term (xv st-block stationary)."""
                sl = wv_sl[g]
                for st in range(st0, st0 + nst):
                    ps = paps.tile([128, 512], F32, tag="pa",
                                   name=f"vps_{g}_{st}")
                    dr_chain(
                        ps,
                        lambda tt, xj: xv_sb[:, tt, xj, st * 128:(st + 1) * 128],
                        lambda tt, wj, half: sl[
                            :, tt, wj, half * 256:(half + 1) * 256],
                        NP_DKV,
                    )
                    nc.vector.tensor_copy(
                        out=v4[g][:, st, :].rearrange(
                            "p (h c) -> p h c", c=VROW)[:, :, 0:HD],
                        in_=ps.rearrange("p (h c) -> p h c", c=HD),
                    )

            PT = {}

            def scores_pair(h, s0):
                """two score tiles s0, s0+1: scoresT[sk,sq] -> exp -> pt."""
                for sk in (s0, s0 + 1):
                    pt_sk = pts.tile([128, S], FP16, tag="pt",
                                     name=f"pt{h}_{sk}")
                    PT.setdefault(h, {})[sk] = pt_sk
                    ss = ssps.tile([128, S], F32, tag="ss",
                                   name=f"ss_{h}_{sk}")
                    for c in range(2):
                        nc.tensor.matmul(
                            ss[:, c * 512:(c + 1) * 512],
                            kT[h][:, sk * 128:(sk + 1) * 128],
                            qT[h][:, c * 512:(c + 1) * 512],
                            start=True,
                            stop=True,
                        )
                    nc.scalar.activation(
                        out=pt_sk, in_=ss, func=ACT_EXP, bias=0.0, scale=1.0)

            def pv(h):
                pt_t = PT[h]
                g, hig = h // 4, h % 4
                ao_h = aohp.tile([128, NT_S, HD], FP16, tag="aoh",
                                 name=f"aoh_{h}")
                # 4 rotating accumulator slots: 2 op-pool tiles + both banks
                # of one ss tile (scores(h) has fully drained through exp by
                # now) - wide enough that the recip+scale drain never stalls
                # the PV matmul stream.
                ssa = ssps.tile([128, S], F32, tag="ss", name=f"pvss_{h}")
                opa = opps.tile([128, 512], F32, tag="op", name=f"opa_{h}")
                opb = opps.tile([128, 512], F32, tag="op", name=f"opb_{h}")
                chunk4 = [ssa[:, 0:512], ssa[:, 512:1024], opa, opb]
                for st in range(NT_S):
                    op = chunk4[st % 4]
                    for sk in range(NT_S):
                        nc.tensor.matmul(
                            op[:, 0:VROW],
                            pt_t[sk][:, st * 128:(st + 1) * 128],
                            v4[g][:, sk, hig * VROW:(hig + 1) * VROW],
                            start=(sk == 0),
                            stop=(sk == NT_S - 1),
                        )
                    rec = small.tile([128, 1], F32, tag="rec",
                                     name=f"rec_{h}_{st}")
                    nc.vector.reciprocal(out=rec, in_=op[:, HD:VROW])
                    nc.vector.tensor_scalar_mul(
                        ao_h[:, st, :], op[:, 0:HD], rec)
                # fp16 transpose staging, then fp8 hi/lo planes of 16*ao
                # (both quantize passes on DVE: the ACT queue is the exp
                # critical path)
                aost = aostp.tile([128, S], FP16, tag="aost",
                                  name=f"aost_{h}")
                nc.sync.dma_start_transpose(
                    out=aost.rearrange("p (t c) -> p t c", c=128),
                    in_=ao_h)
                nc.vector.tensor_copy(out=aoT8[:, h, 0, :], in_=aost)
                nc.vector.tensor_tensor(
                    out=aoT8[:, h, 1, :], in0=aost, in1=aoT8[:, h, 0, :],
                    op=SUB)

            # ---- prologue compute, pair-outer interleaved so PE
            # consumption stays behind the DMA arrival rate ----
            def prologue_proj(wsl, x_sb, npair, bias_sb, scl, dst, nm):
                """head-0 projection, contraction-pair-outer (stream order)."""
                dst[0] = qkp.tile([128, S], FP16, tag=nm, name=f"{nm}_0")
                sst = ssps.tile([128, S], F32, tag="ss", name=f"{nm}p")
                pss = [sst[:, 0:512], sst[:, 512:1024]]
                n_tot = npair * 3
                for u in range(npair):
                    tt = slice(2 * u, 2 * u + 2)
                    for ti, (wj, xj) in enumerate(TERMS):
                        n0 = u * 3 + ti
                        for c in range(2):
                            for half in range(2):
                                nc.tensor.matmul(
                                    pss[c][:, half * 256:(half + 1) * 256],
                                    wsl[:, tt, wj, 0:128],
                                    x_sb[:, tt, xj,
                                         c * 512 + half * 256:
                                         c * 512 + half * 256 + 256],
                                    start=(n0 == 0 and half == 0),
                                    stop=(n0 == n_tot - 1 and half == 1),
                                    perf_mode=DR,
                                )
                for c in range(2):
                    nc.scalar.activation(
                        out=dst[0][:, c * 512:(c + 1) * 512], in_=pss[c],
                        func=ACT_IDENT, bias=bias_sb[:, 0:1], scale=scl)

            prologue_proj(wk_sl[0], xk_sb, NP_DKV, bk_sb, KS, kT, "kT")
            prologue_proj(wq_sl[0], xq_sb, NP_DQ, bq_sb, QS, qT, "qT")

            def opull_pairs(u0, u1):
                # out-projection pull-in: chunks st=4,5 of pass 0, pairs
                # u0..u1-1, into the carried pa psum tiles.
                opull = _CARRY["opull"]
                for u in range(u0, u1):
                    tt = slice(2 * u, 2 * u + 2)
                    for i in range(2):  # st = 4 + i
                        st = 4 + i
                        for half in range(2):
                            for wj, xj in TERMS:
                                nc.tensor.matmul(
                                    opull[i][:, half * 256:(half + 1) * 256],
                                    aoT8[:, tt, wj, st * 128:(st + 1) * 128],
                                    wo_pre[u][:, :, xj,
                                              half * 256:(half + 1) * 256],
                                    start=(u == 0 and half == 0
                                           and (wj, xj) == (0, 0)),
                                    stop=False,
                                    perf_mode=DR,
                                )

            # ---- main loop over heads; iter h projects head h+1,
            # computes scores pairs (h,2..6) + (h+1,0), then pv(h) ----
            for h in range(_DEV_HEADS):
                if h == 0:
                    # xv/wv deferred out of the prologue DMA critical path;
                    # wv0 + xv are needed by iter 0's vproj slot, wv1 later
                    load_wv(0)
                    stream_x(xv_sb, xv_r, NT_DKV)
                    load_wv(1)
                if h % 2 == 0 and (h + 2) // 2 < H // 2:
                    load_wqk2((h + 2) // 2)
                if h == 3:
                    load_wv(2)
                elif h == 7:
                    load_wv(3)
                elif h == 11:
                    # prefetch pass-0 out-projection pair-tiles (parked
                    # until iters 14/15 + the epilogue)
                    for u in range(6):
                        wo_t = wop.tile([128, 2, 2, 512], FP8, tag="wo",
                                        name=f"wo_0_{u}")
                        nc.scalar.dma_start(
                            out=wo_t, in_=wo_r[:, 2 * u:2 * u + 2, :, 0:512])
                        wo_pre.append(wo_t)

                vg, vst0, vnst = None, 0, 0
                if h == 0:
                    vg, vst0, vnst = 0, 0, 8
                elif h <= 12:
                    vg, vst0, vnst = (h + 3) // 4, ((h - 1) % 4) * 2, 2
                if vnst and vst0 == 0:
                    v4_alloc(vg)

                if h == 0:
                    # iter 0 choreography: head-1 projections reuse weight
                    # slice 0 (no DMA dependency), so they + head-0 scores
                    # keep the PE ramped while wv0/xv stream in; vproj g0
                    # then runs at full speed before pv(0).  scores stay
                    # interleaved so the ACT exp drain keeps pace.
                    qproj(1)
                    scores_pair(0, 0)
                    kproj(1)
                    scores_pair(0, 2)
                    vproj_part(0, 0, 2)
                    scores_pair(0, 4)
                    vproj_part(0, 2, 2)
                    scores_pair(0, 6)
                    vproj_part(0, 4, 2)
                    scores_pair(1, 0)
                    vproj_part(0, 6, 2)
                elif h < H - 1:
                    scores_pair(h, 2)
                    qproj(h + 1)
                    scores_pair(h, 4)
                    kproj(h + 1)
                    scores_pair(h, 6)
                    if vnst:
                        vproj_part(vg, vst0, vnst)
                    if h == H - 2:
                        # iter 14 has no vproj: fill with out-proj pull-in
                        _CARRY["opull"] = [
                            paps.tile([128, 512], F32, tag="pa",
                                      name=f"opull_{i}") for i in range(2)]
                        opull_pairs(0, 3)
                    scores_pair(h + 1, 0)
                else:
                    # iter 15: no projection work; hide exp under pull-in
                    scores_pair(h, 2)
                    opull_pairs(3, 4)
                    scores_pair(h, 4)
                    opull_pairs(4, 5)
                    scores_pair(h, 6)
                    opull_pairs(5, 6)
                pv(h)

            # ---- output projection: out[s, e2] = aoT8.T @ wo8 * OS ----
            # 4 column passes of 8 chunks; wo pair-tiles streamed.
            for e2c in range(_DEV_EPI_PASSES):
                chunks = []
                for i in range(2):
                    t = ssps.tile([128, S], F32, tag="ss", name=f"oss_{e2c}_{i}")
                    chunks.append(t[:, 0:512])
                    chunks.append(t[:, 512:1024])
                for i in range(2):
                    if e2c == 0:
                        chunks.append(_CARRY["opull"][i])
                    else:
                        chunks.append(paps.tile([128, 512], F32, tag="pa",
                                                name=f"opa_{e2c}_{i}"))
                for i in range(2):
                    chunks.append(opps.tile([128, 512], F32, tag="op",
                                            name=f"oop_{e2c}_{i}"))

                def get_wo(u):
                    if e2c == 0 and u < len(wo_pre):
                        return wo_pre[u]
                    wo_t = wop.tile([128, 2, 2, 512], FP8, tag="wo",
                                    name=f"wo_{e2c}_{u}")
                    nc.gpsimd.dma_start(
                        out=wo_t,
                        in_=wo_r[:, 2 * u:2 * u + 2, :,
                                 e2c * 512:(e2c + 1) * 512])
                    return wo_t

                def acc(u, st, wo_t, start, stop):
                    tt = slice(2 * u, 2 * u + 2)
                    for half in range(2):
                        for ti, (wj, xj) in enumerate(TERMS):
                            nc.tensor.matmul(
                                chunks[st][:, half * 256:(half + 1) * 256],
                                aoT8[:, tt, wj, st * 128:(st + 1) * 128],
                                wo_t[:, :, xj,
                                     half * 256:(half + 1) * 256],
                                start=(start and half == 0 and ti == 0),
                                stop=(stop and half == 1 and ti == 2),
                                perf_mode=DR,
                            )

                # pair-major streaming phase (u 0..3)
                for u in range(4):
                    wo_t = get_wo(u)
                    for st in range(NT_S):
                        if e2c == 0 and st in (4, 5):
                            continue  # accumulated during iters 14/15
                        acc(u, st, wo_t, start=(u == 0), stop=False)
                # staggered tail: each chunk finishes its last 4 pairs, then
                # copy+store immediately so completions pipeline out at a
                # spacing above the copy+DMA drain rate
                wo_tail = {u: get_wo(u) for u in range(4, NP_E)}
                for st in range(NT_S):
                    for u in range(4, NP_E):
                        if e2c == 0 and st in (4, 5) and u < 6:
                            continue  # pairs 0..5 pulled into iters 14/15
                        acc(u, st, wo_tail[u], start=False,
                            stop=(u == NP_E - 1))
                    ot = outsb.tile([128, 512], BF16, tag="outt",
                                    name=f"ot_{e2c}_{st}")
                    if st % 2 == 0:
                        nc.vector.tensor_scalar_mul(ot, chunks[st], OS)
                        eng = nc.sync
                    else:
                        nc.scalar.mul(ot, chunks[st], OS)
                        eng = nc.scalar
                    eng.dma_start(
                        out=out.ap()[st * 128:(st + 1) * 128,
                                     e2c * 512:(e2c + 1) * 512],
                        in_=ot,
                    )

    nc.compile()
    return nc


def _get_nc():
    if "nc" not in _CACHED:
        _CACHED["nc"] = _build()
    return _CACHED["nc"]


def _numpy_reference(query, key, value, attention_mask,
                     Wq, bq, Wk, bk, Wv, bv, Wo, bo):
    # general fallback (only used when attention_mask isn't all ones)
    Bb, SQ, _ = query.shape
    SK = key.shape[1]
    q = query @ Wq.T + bq
    k = key @ Wk.T + bk
    v = value @ Wv.T + bv
    q = q.reshape(Bb, SQ, H, HD).transpose(0, 2, 1, 3)
    k = k.reshape(Bb, SK, H, HD).transpose(0, 2, 1, 3)
    v = v.reshape(Bb, SK, H, HD).transpose(0, 2, 1, 3)
    scores = np.einsum("bhqd,bhkd->bhqk", q, k) * (HD ** -0.5)
    scores = np.where(attention_mask[:, None, :, :] == 0,
                      np.float32(-1e10), scores)
    scores -= scores.max(-1, keepdims=True)
    p = np.exp(scores)
    p /= p.sum(-1, keepdims=True)
    o = np.einsum("bhqk,bhkd->bhqd", p, v)
    o = o.transpose(0, 2, 1, 3).reshape(Bb, SQ, E)
    return (o @ Wo.T + bo).astype(np.float32)


def _hilo(aT):
    """[K, C] f32 -> [2K, C] fp8 rows in (t j p) block-interleaved order."""
    f8 = ml_dtypes.float8_e4m3
    K, C = aT.shape
    hi = aT.astype(f8)
    lo = (aT - hi.astype(np.float32)).astype(f8)
    a = np.stack([hi.reshape(K // 128, 128, C),
                  lo.reshape(K // 128, 128, C)], axis=1)  # [t, j, p, C]
    return np.ascontiguousarray(a.reshape(2 * K, C))


def _hilo_sliced(aT, slice_cols):
    """[K, C] f32 -> [nsl*128, nt*2*slice_cols] fp8, partition-major
    per-slice layout: row (sl p), cols (t j c)."""
    f8 = ml_dtypes.float8_e4m3
    K, C = aT.shape
    nt, nsl = K // 128, C // slice_cols
    hi = aT.astype(f8)
    lo = (aT - hi.astype(np.float32)).astype(f8)
    a = np.stack([hi.reshape(nt, 128, C), lo.reshape(nt, 128, C)],
                 axis=1)                    # [t, j, p, C]
    a = a.reshape(nt, 2, 128, nsl, slice_cols)
    a = a.transpose(3, 2, 0, 1, 4)          # [sl, p, t, j, c]
    return np.ascontiguousarray(a.reshape(nsl * 128, nt * 2 * slice_cols))


def _prepare_in_maps(inputs):
    query = np.asarray(inputs["query"], dtype=np.float32)
    key = np.asarray(inputs["key"], dtype=np.float32)
    value = np.asarray(inputs["value"], dtype=np.float32)
    Wq = np.asarray(inputs["Wq"], dtype=np.float32)
    bq = np.asarray(inputs["bq"], dtype=np.float32)
    Wk = np.asarray(inputs["Wk"], dtype=np.float32)
    bk = np.asarray(inputs["bk"], dtype=np.float32)
    Wv = np.asarray(inputs["Wv"], dtype=np.float32)
    Wo = np.asarray(inputs["Wo"], dtype=np.float32)

    scale = np.float32(HD ** -0.5)
    wq8 = _hilo_sliced(np.ascontiguousarray(Wq.T) * (scale * (1.0 / QS)), 256)
    wk8 = _hilo_sliced(np.ascontiguousarray(Wk.T) * (1.0 / KS), 256)
    wv8 = _hilo_sliced(np.ascontiguousarray(Wv.T) * 32.0, 512)
    wo8 = _hilo(np.ascontiguousarray(Wo.T) * 64.0)
    bq_s = (bq * scale).astype(np.float32)

    in_maps = []
    for b in range(B):
        in_maps.append({
            "xq8": _hilo(np.ascontiguousarray(query[b].T)),
            "xk8": _hilo(np.ascontiguousarray(key[b].T)),
            "xv8": _hilo(np.ascontiguousarray(value[b].T)),
            "wq8": wq8, "wk8": wk8, "wv8": wv8, "wo8": wo8,
            "bq": bq_s, "bk": bk.astype(np.float32),
        })
    return in_maps


def run_on_device(inputs, **spmd_kwargs):
    """Run the bass kernel; returns (out [B,S,E] f32, BassKernelResults)."""
    in_maps = _prepare_in_maps(inputs)
    Wo = np.asarray(inputs["Wo"], dtype=np.float64)
    bv = np.asarray(inputs["bv"], dtype=np.float64)
    bo = np.asarray(inputs["bo"], dtype=np.float64)
    bo_eff = (Wo @ bv + bo).astype(np.float32)
    res = run_bass_kernel_spmd(_get_nc(), in_maps,
                               core_ids=list(range(B)), **spmd_kwargs)
    out = np.stack([res.results[b]["out"].astype(np.float32)
                    for b in range(B)], axis=0)
    return (out + bo_eff).astype(np.float32), res


def kernel(**inputs):
    mask = np.asarray(inputs["attention_mask"])
    if not mask.all():
        return _numpy_reference(
            np.asarray(inputs["query"], dtype=np.float32),
            np.asarray(inputs["key"], dtype=np.float32),
            np.asarray(inputs["value"], dtype=np.float32), mask,
            np.asarray(inputs["Wq"], dtype=np.float32),
            np.asarray(inputs["bq"], dtype=np.float32),
            np.asarray(inputs["Wk"], dtype=np.float32),
            np.asarray(inputs["bk"], dtype=np.float32),
            np.asarray(inputs["Wv"], dtype=np.float32),
            np.asarray(inputs["bv"], dtype=np.float32),
            np.asarray(inputs["Wo"], dtype=np.float32),
            np.asarray(inputs["bo"], dtype=np.float32))
    out, _ = run_on_device(inputs)
    return out


# revision 59
# speedup vs baseline: 1.0002x; 1.0002x over previous
"""CrossAttention kernel for 8 Trainium2 NeuronCores.

Problem (hardcoded): B=8, SQ=SK=1024, Q_DIM=2048, KV_DIM=1024, E_DIM=2048,
H=16 heads, HD=128.  out = softmax((X_q Wq^T + bq)(X_k Wk^T + bk)^T / sqrt(HD))
                            @ (X_v Wv^T + bv) @ Wo^T + bo

Sharding: data-parallel over batch - each of the 8 cores computes one batch
element end-to-end; no collectives.

Per-core dataflow: the four big projections (q/k/v/out) run as fp8-e4m3
DoubleRow matmuls with hi+lo residual splitting: every operand X is stored
as X_hi = fp8(X), X_lo = fp8(X - X_hi), and each pair of 128-row
contraction blocks is covered by 3 DoubleRow instructions
  (W_hi.X_hi), (W_lo.X_hi), (W_hi.X_lo)   [W_lo.X_lo ~ 0.1% dropped]
giving 0.75x the bf16 cycle count at bf16-grade accuracy.  Host-side
operands (x inputs, all weights) are split for free; `ao` is split on
device after the PV transpose.  scores and PV stay fp16 (contraction is
only HD=128 there, DoubleRow pairing has nothing to pair).

Power-of-2 pre-scales keep every fp8 tensor's magnitude in e4m3's normal
range; they are undone via ACT scale params, the PV ones-column, and the
final output copy scale:
  wq' = Wq^T * HD^-0.5 * 2^9   -> qT = psum * 2^-9 + bq * HD^-0.5
  wk' = Wk^T * 2^5             -> kT = psum * 2^-5 + bk
  wv' = Wv^T * 2^5             -> v4 holds v * 32 (fp16)
  ones column = 2.0            -> ao = 16 * pv / sum(p)   (fp8-friendly)
  wo' = Wo^T * 2^6             -> out = psum * 2^-10
bv is folded into bo on the host (softmax rows sum to 1).

Software pipeline: iter h computes scores pairs (h,2..6)+(h+1,0)
interleaved with projections for head h+1 (one-iteration lead; the
shifted score schedule keeps the ACT exp drain ahead of pv(h), which
keeps the PE busy and avoids cost-model p-state ramp resets), then
PV(h) -> ao -> fp16 DMA-transpose -> fp8 hi/lo quantize into aoT8.
The prologue streams only wk0/xk/wq0/xq (the DMA device is one serial
resource) and computes head-0 q/k projections pair-outer behind the
stream; xv/wv and all of vproj group 0 live in iter 0, whose head-1
projections reuse weight slice 0 and need no new DMA.  Iters 14/15 pull
in out-projection accumulation (chunks st=4,5 of pass 0, pairs 0..5) to
hide the exp drain; the epilogue runs the remaining out-projection with
streamed Wo pair-tiles and staggered chunk drains.
"""

import sys

sys.path.insert(0, "/opt/trn_rl_repo")

import numpy as np
import ml_dtypes

import concourse.tile as tile
from concourse import bacc
import concourse.mybir as mybir
from concourse.bass_utils import run_bass_kernel_spmd

F32 = mybir.dt.float32
BF16 = mybir.dt.bfloat16
FP16 = mybir.dt.float16
FP8 = mybir.dt.float8e4
DR = mybir.MatmulPerfMode.DoubleRow
ACT_IDENT = mybir.ActivationFunctionType.Identity
ACT_COPY = mybir.ActivationFunctionType.Copy
ACT_EXP = mybir.ActivationFunctionType.Exp
SUB = mybir.AluOpType.subtract

B = 8
S = 1024          # SQ == SK
DQ = 2048         # query input dim
DKV = 1024        # key/value input dim
E = 2048          # embed dim
H = 16            # heads
HD = 128          # head dim
NT_S = S // 128   # 8 seq tiles
NT_E = E // 128   # 16 e tiles (== heads)
NT_DQ = DQ // 128
NT_DKV = DKV // 128
NP_DQ = NT_DQ // 2   # 8 contraction block-pairs
NP_DKV = NT_DKV // 2  # 4
NP_E = NT_E // 2     # 8
VROW = HD + 1     # head block in v group incl. ones column

# power-of-2 scale folding (see module docstring)
QS = 2.0 ** -9
KS = 2.0 ** -5
ONES_V = 2.0
OS = 2.0 ** -10
# the 3 DoubleRow terms per contraction block-pair: (w_j, x_j) digit picks
TERMS = ((0, 0), (1, 0), (0, 1))

_CACHED = {}
_DEV_EPI_PASSES = 4   # dev knob: number of epilogue passes to emit
_DEV_HEADS = H        # dev knob: number of main-loop head iterations


def _build():
    nc = bacc.Bacc("TRN2", target_bir_lowering=False, debug=False)

    xq8 = nc.dram_tensor("xq8", [2 * DQ, S], FP8, kind="ExternalInput")
    xk8 = nc.dram_tensor("xk8", [2 * DKV, S], FP8, kind="ExternalInput")
    xv8 = nc.dram_tensor("xv8", [2 * DKV, S], FP8, kind="ExternalInput")
    # weights arrive pre-sliced in partition-major layout so each slice
    # load is one full-rate DMA (>=512B contiguous per partition)
    wq8 = nc.dram_tensor("wq8", [8 * 128, NT_DQ * 2 * 256], FP8,
                         kind="ExternalInput")
    wk8 = nc.dram_tensor("wk8", [8 * 128, NT_DKV * 2 * 256], FP8,
                         kind="ExternalInput")
    wv8 = nc.dram_tensor("wv8", [4 * 128, NT_DKV * 2 * 512], FP8,
                         kind="ExternalInput")
    wo8 = nc.dram_tensor("wo8", [2 * E, E], FP8, kind="ExternalInput")
    bq = nc.dram_tensor("bq", [E], F32, kind="ExternalInput")
    bk = nc.dram_tensor("bk", [E], F32, kind="ExternalInput")
    out = nc.dram_tensor("out", [S, E], BF16, kind="ExternalOutput")

    xq_r = xq8.rearrange("(t j p) s -> p t j s", p=128, j=2)
    xk_r = xk8.rearrange("(t j p) s -> p t j s", p=128, j=2)
    xv_r = xv8.rearrange("(t j p) s -> p t j s", p=128, j=2)
    wq_r = wq8.rearrange("(sl p) (t jj c) -> sl p t jj c",
                         p=128, jj=2, c=256)
    wk_r = wk8.rearrange("(sl p) (t jj c) -> sl p t jj c",
                         p=128, jj=2, c=256)
    wv_r = wv8.rearrange("(sl p) (t jj c) -> sl p t jj c",
                         p=128, jj=2, c=512)
    wo_r = wo8.rearrange("(t j p) e -> p t j e", p=128, j=2)

    with tile.TileContext(nc) as tc:
        with (
            tc.tile_pool(name="persist", bufs=1) as persist,
            tc.tile_pool(name="qk", bufs=2) as qkp,
            tc.tile_pool(name="v4p", bufs=2) as v4p,
            tc.tile_pool(name="pts", bufs=10) as pts,
            tc.tile_pool(name="aohp", bufs=2) as aohp,
            tc.tile_pool(name="aostp", bufs=2) as aostp,
            tc.tile_pool(name="wqp", bufs=2) as wqp,
            tc.tile_pool(name="wkp", bufs=2) as wkp,
            tc.tile_pool(name="wvp", bufs=2) as wvp,
            tc.tile_pool(name="wop", bufs=7) as wop,
            tc.tile_pool(name="outsb", bufs=4) as outsb,
            tc.tile_pool(name="small", bufs=4) as small,
            tc.tile_pool(name="ssps", bufs=2, space="PSUM") as ssps,
            tc.tile_pool(name="paps", bufs=2, space="PSUM") as paps,
            tc.tile_pool(name="opps", bufs=2, space="PSUM") as opps,
        ):
            # ---- resident inputs / constants ----
            bq_sb = persist.tile([128, NT_E], F32, tag="bq")
            bk_sb = persist.tile([128, NT_E], F32, tag="bk")
            nc.gpsimd.dma_start(out=bq_sb, in_=bq.rearrange("(t p) -> p t", p=128))
            nc.gpsimd.dma_start(out=bk_sb, in_=bk.rearrange("(t p) -> p t", p=128))

            xq_sb = persist.tile([128, NT_DQ, 2, S], FP8, tag="xq")
            xk_sb = persist.tile([128, NT_DKV, 2, S], FP8, tag="xk")
            xv_sb = persist.tile([128, NT_DKV, 2, S], FP8, tag="xv")
            aoT8 = persist.tile([128, NT_E, 2, S], FP8, tag="aoT8")

            wq_sl = {}
            wk_sl = {}
            wv_sl = {}

            def load_wqk2(j, eng=None):  # heads 2j, 2j+1
                eng = eng or nc.scalar
                wq_sl[j] = wqp.tile([128, NT_DQ, 2, 256], FP8, tag="wq",
                                    name=f"wq2_{j}")
                eng.dma_start(out=wq_sl[j], in_=wq_r[j])
                wk_sl[j] = wkp.tile([128, NT_DKV, 2, 256], FP8, tag="wk",
                                    name=f"wk2_{j}")
                eng.dma_start(out=wk_sl[j], in_=wk_r[j])

            def load_wv(g, eng=None):  # heads 4g..4g+3
                eng = eng or nc.scalar
                wv_sl[g] = wvp.tile([128, NT_DKV, 2, 512], FP8, tag="wv",
                                    name=f"wv_{g}")
                eng.dma_start(out=wv_sl[g], in_=wv_r[g])

            # Prologue DMAs: only what the prologue compute needs (the DMA
            # device is a single serial resource in practice) — wk0, xk,
            # wq0, xq.  xv/wv stream during iter 0, whose vproj slot runs
            # all of group 0.  Input streams split across sync (even t) and
            # gpsimd (odd t) queues in consumption order; weight slices on
            # the scalar queue.
            wk_sl[0] = wkp.tile([128, NT_DKV, 2, 256], FP8, tag="wk",
                                name="wk2_0")
            nc.scalar.dma_start(out=wk_sl[0][:, 0:4], in_=wk_r[0][:, 0:4])
            nc.scalar.dma_start(out=wk_sl[0][:, 4:8], in_=wk_r[0][:, 4:8])

            def stream_x(sb, r_, nt):
                for t0 in range(0, nt, 4):
                    nc.sync.dma_start(out=sb[:, t0:t0 + 4],
                                      in_=r_[:, t0:t0 + 4])

            stream_x(xk_sb, xk_r, NT_DKV)
            load_wv(0)
            stream_x(xv_sb, xv_r, NT_DKV)
            wq_sl[0] = wqp.tile([128, NT_DQ, 2, 256], FP8, tag="wq",
                                name="wq2_0")
            nc.scalar.dma_start(out=wq_sl[0][:, 0:8], in_=wq_r[0][:, 0:8])
            nc.scalar.dma_start(out=wq_sl[0][:, 8:16], in_=wq_r[0][:, 8:16])
            stream_x(xq_sb, xq_r, NT_DQ)
            wo_pre = []

            qT = {}
            kT = {}
            v4 = {}
            _CARRY = {}

            def dr_chain(ps, stat_fn, mov_fn, npair):
                """48/24-instr 3-term DoubleRow chain into psum [128,512].

                stat_fn(tt, j) / mov_fn(tt, j, half) give the operand APs.
                One accumulation group: start on the first instr, stop on
                the last (the 2KB psum region lazily zeroes, so the second
                256-col half accumulates from clean zero).
                """
                n_tot = 2 * npair * 3
                n = 0
                for half in range(2):
                    for u in range(npair):
                        tt = slice(2 * u, 2 * u + 2)
                        for wj, xj in TERMS:
                            nc.tensor.matmul(
                                ps[:, half * 256:(half + 1) * 256],
                                stat_fn(tt, wj),
                                mov_fn(tt, xj, half),
                                start=(n == 0),
                                stop=(n == n_tot - 1),
                                perf_mode=DR,
                            )
                            n += 1

            def qproj(h):
                """qT[h] [e128, s] fp16 <- fp8 DR 3-term over DQ."""
                sl = wq_sl[h // 2]
                hoff = (h % 2) * 128
                qT[h] = qkp.tile([128, S], FP16, tag="qT", name=f"qT_{h}")
                for c in range(2):
                    ps = paps.tile([128, 512], F32, tag="pa", name=f"qps_{h}_{c}")
                    dr_chain(
                        ps,
                        lambda tt, wj: sl[:, tt, wj, hoff:hoff + 128],
                        lambda tt, xj, half: xq_sb[
                            :, tt, xj, c * 512 + half * 256:c * 512 + half * 256 + 256],
                        NP_DQ,
                    )
                    nc.scalar.activation(
                        out=qT[h][:, c * 512:(c + 1) * 512], in_=ps,
                        func=ACT_IDENT, bias=bq_sb[:, h:h + 1], scale=QS)

            def kproj(h):
                sl = wk_sl[h // 2]
                hoff = (h % 2) * 128
                kT[h] = qkp.tile([128, S], FP16, tag="kT", name=f"kT_{h}")
                for c in range(2):
                    ps = paps.tile([128, 512], F32, tag="pa", name=f"kps_{h}_{c}")
                    dr_chain(
                        ps,
                        lambda tt, wj: sl[:, tt, wj, hoff:hoff + 128],
                        lambda tt, xj, half: xk_sb[
                            :, tt, xj, c * 512 + half * 256:c * 512 + half * 256 + 256],
                        NP_DKV,
                    )
                    nc.scalar.activation(
                        out=kT[h][:, c * 512:(c + 1) * 512], in_=ps,
                        func=ACT_IDENT, bias=bk_sb[:, h:h + 1], scale=KS)

            def v4_alloc(g):
                v4[g] = v4p.tile([128, NT_S, 4 * VROW], FP16, tag="v4",
                                 name=f"v4_{g}")
                nc.vector.memset(
                    v4[g].rearrange("p t (h c) -> p t h c", c=VROW)
                    [:, :, :, HD:], ONES_V)

            def vproj_part(g, st0, nst):
                """v4[g][:, st, :] <- DR 3-term (xv st-block stationary)."""
                sl = wv_sl[g]
                for st in range(st0, st0 + nst):
                    ps = paps.tile([128, 512], F32, tag="pa",
                                   name=f"vps_{g}_{st}")
                    dr_chain(
                        ps,
                        lambda tt, xj: xv_sb[:, tt, xj, st * 128:(st + 1) * 128],
                        lambda tt, wj, half: sl[
                            :, tt, wj, half * 256:(half + 1) * 256],
                        NP_DKV,
                    )
                    nc.vector.tensor_copy(
                        out=v4[g][:, st, :].rearrange(
                            "p (h c) -> p h c", c=VROW)[:, :, 0:HD],
                        in_=ps.rearrange("p (h c) -> p h c", c=HD),
                    )

            PT = {}

            def scores_pair(h, s0):
                """two score tiles s0, s0+1: scoresT[sk,sq] -> exp -> pt."""
                for sk in (s0, s0 + 1):
                    pt_sk = pts.tile([128, S], FP16, tag="pt",
                                     name=f"pt{h}_{sk}")
                    PT.setdefault(h, {})[sk] = pt_sk
                    ss = ssps.tile([128, S], F32, tag="ss",
                                   name=f"ss_{h}_{sk}")
                    for c in range(2):
                        nc.tensor.matmul(
                            ss[:, c * 512:(c + 1) * 512],
                            kT[h][:, sk * 128:(sk + 1) * 128],
                            qT[h][:, c * 512:(c + 1) * 512],
                            start=True,
                            stop=True,
                        )
                    nc.scalar.activation(
                        out=pt_sk, in_=ss, func=ACT_EXP, bias=0.0, scale=1.0)

            def pv(h):
                pt_t = PT[h]
                g, hig = h // 4, h % 4
                ao_h = aohp.tile([128, NT_S, HD], FP16, tag="aoh",
                                 name=f"aoh_{h}")
                # 4 rotating accumulator slots: 2 op-pool tiles + both banks
                # of one ss tile (scores(h) has fully drained through exp by
                # now) - wide enough that the recip+scale drain never stalls
                # the PV matmul stream.
                ssa = ssps.tile([128, S], F32, tag="ss", name=f"pvss_{h}")
                opa = opps.tile([128, 512], F32, tag="op", name=f"opa_{h}")
                opb = opps.tile([128, 512], F32, tag="op", name=f"opb_{h}")
                chunk4 = [ssa[:, 0:512], ssa[:, 512:1024], opa, opb]
                for st in range(NT_S):
                    op = chunk4[st % 4]
                    for sk in range(NT_S):
                        nc.tensor.matmul(
                            op[:, 0:VROW],
                            pt_t[sk][:, st * 128:(st + 1) * 128],
                            v4[g][:, sk, hig * VROW:(hig + 1) * VROW],
                            start=(sk == 0),
                            stop=(sk == NT_S - 1),
                        )
                    rec = small.tile([128, 1], F32, tag="rec",
                                     name=f"rec_{h}_{st}")
                    nc.vector.reciprocal(out=rec, in_=op[:, HD:VROW])
                    nc.vector.tensor_scalar_mul(
                        ao_h[:, st, :], op[:, 0:HD], rec)
                # fp16 transpose staging, then fp8 hi/lo planes of 16*ao
                # (both quantize passes on DVE: the ACT queue is the exp
                # critical path)
                aost = aostp.tile([128, S], FP16, tag="aost",
                                  name=f"aost_{h}")
                nc.sync.dma_start_transpose(
                    out=aost.rearrange("p (t c) -> p t c", c=128),
                    in_=ao_h)
                nc.vector.tensor_copy(out=aoT8[:, h, 0, :], in_=aost)
                nc.vector.tensor_tensor(
                    out=aoT8[:, h, 1, :], in0=aost, in1=aoT8[:, h, 0, :],
                    op=SUB)

            # ---- prologue compute, pair-outer interleaved so PE
            # consumption stays behind the DMA arrival rate ----
            def prologue_proj(wsl, x_sb, npair, bias_sb, scl, dst, nm):
                """head-0 projection, contraction-pair-outer (stream order)."""
                dst[0] = qkp.tile([128, S], FP16, tag=nm, name=f"{nm}_0")
                sst = ssps.tile([128, S], F32, tag="ss", name=f"{nm}p")
                pss = [sst[:, 0:512], sst[:, 512:1024]]
                n_tot = npair * 3
                for u in range(npair):
                    tt = slice(2 * u, 2 * u + 2)
                    for ti, (wj, xj) in enumerate(TERMS):
                        n0 = u * 3 + ti
                        for c in range(2):
                            for half in range(2):
                                nc.tensor.matmul(
                                    pss[c][:, half * 256:(half + 1) * 256],
                                    wsl[:, tt, wj, 0:128],
                                    x_sb[:, tt, xj,
                                         c * 512 + half * 256:
                                         c * 512 + half * 256 + 256],
                                    start=(n0 == 0 and half == 0),
                                    stop=(n0 == n_tot - 1 and half == 1),
                                    perf_mode=DR,
                                )
                for c in range(2):
                    nc.scalar.activation(
                        out=dst[0][:, c * 512:(c + 1) * 512], in_=pss[c],
                        func=ACT_IDENT, bias=bias_sb[:, 0:1], scale=scl)

            prologue_proj(wk_sl[0], xk_sb, NP_DKV, bk_sb, KS, kT, "kT")
            # vproj g0 in the prologue, contraction-pair-outer so it rides
            # the xv stream and its 10.2us of PE work then hides the xq
            # stream (instead of the PE idling through it)
            v4_alloc(0)
            for rnd in range(2):
                pv_ps = [paps.tile([128, 512], F32, tag="pa",
                                   name=f"vp{rnd}_{i}") for i in range(2)]
                sst = ssps.tile([128, S], F32, tag="ss", name=f"vp{rnd}ss")
                pv_ps.append(sst[:, 0:512])
                pv_ps.append(sst[:, 512:1024])
                n_tot = NP_DKV * 3
                for u in range(NP_DKV):
                    tt = slice(2 * u, 2 * u + 2)
                    for ti, (wj, xj) in enumerate(TERMS):
                        n0 = u * 3 + ti
                        for i in range(4):
                            st = rnd * 4 + i
                            for half in range(2):
                                nc.tensor.matmul(
                                    pv_ps[i][:, half * 256:(half + 1) * 256],
                                    xv_sb[:, tt, xj,
                                          st * 128:(st + 1) * 128],
                                    wv_sl[0][:, tt, wj,
                                             half * 256:(half + 1) * 256],
                                    start=(n0 == 0 and half == 0),
                                    stop=(n0 == n_tot - 1 and half == 1),
                                    perf_mode=DR,
                                )
                for i in range(4):
                    st = rnd * 4 + i
                    nc.vector.tensor_copy(
                        out=v4[0][:, st, :].rearrange(
                            "p (h c) -> p h c", c=VROW)[:, :, 0:HD],
                        in_=pv_ps[i].rearrange("p (h c) -> p h c", c=HD),
                    )
            prologue_proj(wq_sl[0], xq_sb, NP_DQ, bq_sb, QS, qT, "qT")

            def opull_pairs(u0, u1):
                # out-projection pull-in: chunks st=4,5 of pass 0, pairs
                # u0..u1-1, into the carried pa psum tiles.
                opull = _CARRY["opull"]
                for u in range(u0, u1):
                    tt = slice(2 * u, 2 * u + 2)
                    for i in range(2):  # st = 4 + i
                        st = 4 + i
                        for half in range(2):
                            for wj, xj in TERMS:
                                nc.tensor.matmul(
                                    opull[i][:, half * 256:(half + 1) * 256],
                                    aoT8[:, tt, wj, st * 128:(st + 1) * 128],
                                    wo_pre[u][:, :, xj,
                                              half * 256:(half + 1) * 256],
                                    start=(u == 0 and half == 0
                                           and (wj, xj) == (0, 0)),
                                    stop=False,
                                    perf_mode=DR,
                                )

            # ---- main loop over heads; iter h projects head h+1,
            # computes scores pairs (h,2..6) + (h+1,0), then pv(h) ----
            for h in range(_DEV_HEADS):
                if h == 0:
                    load_wv(1)
                if h % 2 == 0 and (h + 2) // 2 < H // 2:
                    load_wqk2((h + 2) // 2)
                if h == 3:
                    load_wv(2)
                elif h == 7:
                    load_wv(3)
                elif h == 11:
                    # prefetch pass-0 out-projection pair-tiles (parked
                    # until iters 14/15 + the epilogue)
                    for u in range(6):
                        wo_t = wop.tile([128, 2, 2, 512], FP8, tag="wo",
                                        name=f"wo_0_{u}")
                        nc.scalar.dma_start(
                            out=wo_t, in_=wo_r[:, 2 * u:2 * u + 2, :, 0:512])
                        wo_pre.append(wo_t)

                vg, vst0, vnst = None, 0, 0
                if h <= 11:
                    vg, vst0, vnst = h // 4 + 1, (h % 4) * 2, 2
                    if vst0 == 0:
                        v4_alloc(vg)

                if h == 0:
                    # iter 0: everything here runs on data already resident
                    # (head-1 projections reuse weight slice 0), so the
                    # whole iteration is DMA-independent dense PE work while
                    # wv1/wqk2(1) stream for iter 1.  scores_pair(0,0)
                    # replaces the usual carried-over pair from iter h-1.
                    qproj(1)
                    scores_pair(0, 0)
                    kproj(1)
                    scores_pair(0, 2)
                    vproj_part(1, 0, 2)
                    scores_pair(0, 4)
                    scores_pair(0, 6)
                    scores_pair(1, 0)
                elif h < H - 1:
                    scores_pair(h, 2)
                    qproj(h + 1)
                    scores_pair(h, 4)
                    kproj(h + 1)
                    scores_pair(h, 6)
                    if vnst:
                        vproj_part(vg, vst0, vnst)
                    if h == H - 2:
                        # iter 14 has no vproj: fill with out-proj pull-in
                        _CARRY["opull"] = [
                            paps.tile([128, 512], F32, tag="pa",
                                      name=f"opull_{i}") for i in range(2)]
                        opull_pairs(0, 3)
                    scores_pair(h + 1, 0)
                else:
                    # iter 15: no projection work; hide exp under pull-in
                    scores_pair(h, 2)
                    opull_pairs(3, 4)
                    scores_pair(h, 4)
                    opull_pairs(4, 5)
                    scores_pair(h, 6)
                    opull_pairs(5, 6)
                pv(h)

            # ---- output projection: out[s, e2] = aoT8.T @ wo8 * OS ----
            # 4 column passes of 8 chunks; wo pair-tiles streamed.
            for e2c in range(_DEV_EPI_PASSES):
                chunks = []
                for i in range(2):
                    t = ssps.tile([128, S], F32, tag="ss", name=f"oss_{e2c}_{i}")
                    chunks.append(t[:, 0:512])
                    chunks.append(t[:, 512:1024])
                for i in range(2):
                    if e2c == 0:
                        chunks.append(_CARRY["opull"][i])
                    else:
                        chunks.append(paps.tile([128, 512], F32, tag="pa",
                                                name=f"opa_{e2c}_{i}"))
                for i in range(2):
                    chunks.append(opps.tile([128, 512], F32, tag="op",
                                            name=f"oop_{e2c}_{i}"))

                def get_wo(u):
                    if e2c == 0 and u < len(wo_pre):
                        return wo_pre[u]
                    wo_t = wop.tile([128, 2, 2, 512], FP8, tag="wo",
                                    name=f"wo_{e2c}_{u}")
                    nc.gpsimd.dma_start(
                        out=wo_t,
                        in_=wo_r[:, 2 * u:2 * u + 2, :,
                                 e2c * 512:(e2c + 1) * 512])
                    return wo_t

                def acc(u, st, wo_t, start, stop):
                    tt = slice(2 * u, 2 * u + 2)
                    for half in range(2):
                        for ti, (wj, xj) in enumerate(TERMS):
                            nc.tensor.matmul(
                                chunks[st][:, half * 256:(half + 1) * 256],
                                aoT8[:, tt, wj, st * 128:(st + 1) * 128],
                                wo_t[:, :, xj,
                                     half * 256:(half + 1) * 256],
                                start=(start and half == 0 and ti == 0),
                                stop=(stop and half == 1 and ti == 2),
                                perf_mode=DR,
                            )

                # pair-major streaming phase (u 0..3)
                for u in range(4):
                    wo_t = get_wo(u)
                    for st in range(NT_S):
                        if e2c == 0 and st in (4, 5):
                            continue  # accumulated during iters 14/15
                        acc(u, st, wo_t, start=(u == 0), stop=False)
                # staggered tail: each chunk finishes its last 4 pairs, then
                # copy+store immediately so completions pipeline out at a
                # spacing above the copy+DMA drain rate
                wo_tail = {u: get_wo(u) for u in range(4, NP_E)}
                for st in range(NT_S):
                    for u in range(4, NP_E):
                        if e2c == 0 and st in (4, 5) and u < 6:
                            continue  # pairs 0..5 pulled into iters 14/15
                        acc(u, st, wo_tail[u], start=False,
                            stop=(u == NP_E - 1))
                    ot = outsb.tile([128, 512], BF16, tag="outt",
                                    name=f"ot_{e2c}_{st}")
                    if st % 2 == 0:
                        nc.vector.tensor_scalar_mul(ot, chunks[st], OS)
                        eng = nc.sync
                    else:
                        nc.scalar.mul(ot, chunks[st], OS)
                        eng = nc.scalar
                    eng.dma_start(
                        out=out.ap()[st * 128:(st + 1) * 128,
                                     e2c * 512:(e2c + 1) * 512],
                        in_=ot,
                    )

    nc.compile()
    return nc


def _get_nc():
    if "nc" not in _CACHED:
        _CACHED["nc"] = _build()
    return _CACHED["nc"]


def _numpy_reference(query, key, value, attention_mask,
                     Wq, bq, Wk, bk, Wv, bv, Wo, bo):
    # general fallback (only used when attention_mask isn't all ones)
    Bb, SQ, _ = query.shape
    SK = key.shape[1]
    q = query @ Wq.T + bq
    k = key @ Wk.T + bk
    v = value @ Wv.T + bv
    q = q.reshape(Bb, SQ, H, HD).transpose(0, 2, 1, 3)
    k = k.reshape(Bb, SK, H, HD).transpose(0, 2, 1, 3)
    v = v.reshape(Bb, SK, H, HD).transpose(0, 2, 1, 3)
    scores = np.einsum("bhqd,bhkd->bhqk", q, k) * (HD ** -0.5)
    scores = np.where(attention_mask[:, None, :, :] == 0,
                      np.float32(-1e10), scores)
    scores -= scores.max(-1, keepdims=True)
    p = np.exp(scores)
    p /= p.sum(-1, keepdims=True)
    o = np.einsum("bhqk,bhkd->bhqd", p, v)
    o = o.transpose(0, 2, 1, 3).reshape(Bb, SQ, E)
    return (o @ Wo.T + bo).astype(np.float32)


def _hilo(aT):
    """[K, C] f32 -> [2K, C] fp8 rows in (t j p) block-interleaved order."""
    f8 = ml_dtypes.float8_e4m3
    K, C = aT.shape
    hi = aT.astype(f8)
    lo = (aT - hi.astype(np.float32)).astype(f8)
    a = np.stack([hi.reshape(K // 128, 128, C),
                  lo.reshape(K // 128, 128, C)], axis=1)  # [t, j, p, C]
    return np.ascontiguousarray(a.reshape(2 * K, C))


def _hilo_sliced(aT, slice_cols):
    """[K, C] f32 -> [nsl*128, nt*2*slice_cols] fp8, partition-major
    per-slice layout: row (sl p), cols (t j c)."""
    f8 = ml_dtypes.float8_e4m3
    K, C = aT.shape
    nt, nsl = K // 128, C // slice_cols
    hi = aT.astype(f8)
    lo = (aT - hi.astype(np.float32)).astype(f8)
    a = np.stack([hi.reshape(nt, 128, C), lo.reshape(nt, 128, C)],
                 axis=1)                    # [t, j, p, C]
    a = a.reshape(nt, 2, 128, nsl, slice_cols)
    a = a.transpose(3, 2, 0, 1, 4)          # [sl, p, t, j, c]
    return np.ascontiguousarray(a.reshape(nsl * 128, nt * 2 * slice_cols))


def _prepare_in_maps(inputs):
    query = np.asarray(inputs["query"], dtype=np.float32)
    key = np.asarray(inputs["key"], dtype=np.float32)
    value = np.asarray(inputs["value"], dtype=np.float32)
    Wq = np.asarray(inputs["Wq"], dtype=np.float32)
    bq = np.asarray(inputs["bq"], dtype=np.float32)
    Wk = np.asarray(inputs["Wk"], dtype=np.float32)
    bk = np.asarray(inputs["bk"], dtype=np.float32)
    Wv = np.asarray(inputs["Wv"], dtype=np.float32)
    Wo = np.asarray(inputs["Wo"], dtype=np.float32)

    scale = np.float32(HD ** -0.5)
    wq8 = _hilo_sliced(np.ascontiguousarray(Wq.T) * (scale * (1.0 / QS)), 256)
    wk8 = _hilo_sliced(np.ascontiguousarray(Wk.T) * (1.0 / KS), 256)
    wv8 = _hilo_sliced(np.ascontiguousarray(Wv.T) * 32.0, 512)
    wo8 = _hilo(np.ascontiguousarray(Wo.T) * 64.0)
    bq_s = (bq * scale).astype(np.float32)

    in_maps = []
    for b in range(B):
        in_maps.append({
            "xq8": _hilo(np.ascontiguousarray(query[b].T)),
            "xk8": _hilo(np.ascontiguousarray(key[b].T)),
            "xv8": _hilo(np.ascontiguousarray(value[b].T)),
            "wq8": wq8, "wk8": wk8, "wv8": wv8, "wo8": wo8,
            "bq": bq_s, "bk": bk.astype(np.float32),
        })
    return in_maps


def run_on_device(inputs, **spmd_kwargs):
    """Run the bass kernel; returns (out [B,S,E] f32, BassKernelResults)."""
    in_maps = _prepare_in_maps(inputs)
    Wo = np.asarray(inputs["Wo"], dtype=np.float64)
    bv = np.asarray(inputs["bv"], dtype=np.float64)
    bo = np.asarray(inputs["bo"], dtype=np.float64)
    bo_eff = (Wo @ bv + bo).astype(np.float32)
    res = run_bass_kernel_spmd(_get_nc(), in_maps,
                               core_ids=list(range(B)), **spmd_kwargs)
    out = np.stack([res.results[b]["out"].astype(np.float32)
                    for b in range(B)], axis=0)
    return (out + bo_eff).astype(np.float32), res


def kernel(**inputs):
    mask = np.asarray(inputs["attention_mask"])
    if not mask.all():
        return _numpy_reference(
            np.asarray(inputs["query"], dtype=np.float32),
            np.asarray(inputs["key"], dtype=np.float32),
            np.asarray(inputs["value"], dtype=np.float32), mask,
            np.asarray(inputs["Wq"], dtype=np.float32),
            np.asarray(inputs["bq"], dtype=np.float32),
            np.asarray(inputs["Wk"], dtype=np.float32),
            np.asarray(inputs["bk"], dtype=np.float32),
            np.asarray(inputs["Wv"], dtype=np.float32),
            np.asarray(inputs["bv"], dtype=np.float32),
            np.asarray(inputs["Wo"], dtype=np.float32),
            np.asarray(inputs["bo"], dtype=np.float32))
    out, _ = run_on_device(inputs)
    return out
